# revision 1
# baseline (speedup 1.0000x reference)
"""Trainium2 Bass kernel for nn_DependentLatentModel (HardKuma gated LSTM sampler).

Data-parallel over batch across 8 NeuronCores. Per core:
  phase 1: LU = ln(1 - clip(u, eps, 1-eps))                       (bulk)
  phase 2: XW = scale_g * (x @ Wx_all.T + bias)  (PE transpose + matmul)
  phase 2.5: DRAM round-trip rearranges XW into the loop layouts:
     XWG [30, T*4*BC]  gate pre-acts, groups (i,f,o,g) along free dim
     XAB [1, T*2*BC]   kuma a,b pre-acts on partition 0
  phase 3: T sequential steps; per step a latency-optimized chain using
     only Exp/Ln ACT ops (one table set), DVE arith, and tiny PE matmuls
     accumulating onto ACT-preloaded PSUM tiles.
  phase 4: z = ZB - 0.1 -> DRAM.

Engine constraints honored: compute APs start at partition 0 and all
elementwise ops are partition-aligned, because engines cannot move data
across partitions. Gate groups therefore live on partitions 0:30 and are
separated along the free dim: psG [30, 4*BC] = [i | f | o | g] columns.
The LSTM sigmoid/tanh signs are folded into the weights host-side
(i,f,o rows scaled by -1, g rows by +2) so that
  sigmoid(pre) = 1/(1+exp(pre'))        with pre' = -pre
  tanh(pre)    = 1 - 2/(1+exp(pre'))    with pre' = 2*pre
and every transcendental is Exp/Ln from the natural_log_exp table set:
  softplus(x) = ln(1 + e^x),  x^y = exp(y ln x).

HardKuma clips are folded exactly:
  1/clip(softplus(p), 1e-6, 100) == max(1/softplus(p), 0.01) on reachable
  inputs, and z' := clip(1.2 s, 0.1, 1.1) = z + 0.1, with -0.1*w_z folded
  into the gate bias and the -0.1 shift removed from the output in bulk.
"""

import sys

if "/opt/trn_rl_repo" not in sys.path:
    sys.path.insert(0, "/opt/trn_rl_repo")

from contextlib import ExitStack

import numpy as np

import concourse.bass as bass
import concourse.bass_utils as bass_utils
import concourse.tile as tile
from concourse import bacc, mybir
from concourse._compat import with_exitstack

B, T, D, H = 64, 512, 1536, 30
NCORES = 8
BC = B // NCORES          # batch per core (8)
M = 128                   # phase-2 rows: 4 gate groups at 0/32/64/96 + a,b at 126/127
EPS = 1e-5
LN12 = float(np.log(np.float32(1.2)))
FP32 = mybir.dt.float32
AF = mybir.ActivationFunctionType
OP = mybir.AluOpType

# torch gate order [i, f, g, o] -> our group order (i, f, o, g)
_SRC_GRP = [np.arange(0, 30), np.arange(30, 60), np.arange(90, 120),
            np.arange(60, 90)]
_SCALE_GRP = [-1.0, -1.0, -1.0, 2.0]


@with_exitstack
def _emit(ctx: ExitStack, tc: "tile.TileContext", io: dict, t_len: int):
    nc = tc.nc
    xin = io["xin"]      # [BC, t_len, D]
    uin = io["uin"]      # [BC, t_len]
    wxT = io["wxT"]      # [D, M]    (cols: padded groups + a,b; unscaled)
    wrecT = io["wrecT"]  # [H, 122]  (4x scaled Whh_g.T blocks + wa_h + wb_h)
    wz4 = io["wz4"]      # [1, 120]  (scaled wz per group)
    biasc = io["biasc"]  # [M, 1]    (scaled biases in padded-row layout)
    gscale = io["gscale"]  # [M, 1]  (-1/-1/-1/+2 on group rows, +1 on a,b)
    ident = io["ident"]  # [128, 128]
    zout = io["zout"]    # [BC, t_len]

    n_dc = D // 128
    tchunk = min(128, t_len)
    n_tc = t_len // tchunk
    NW = t_len * BC

    cpool = ctx.enter_context(tc.tile_pool(name="const", bufs=1))

    # ---- persistent tiles ----
    wx_sb = cpool.tile([128, n_dc * M], FP32)
    nc.sync.dma_start(
        wx_sb[:].rearrange("p (k m) -> p k m", k=n_dc),
        wxT.rearrange("(k p) m -> p k m", p=128),
    )
    wrec_sb = cpool.tile([H, 122], FP32)
    nc.sync.dma_start(wrec_sb[:], wrecT)
    wz_sb = cpool.tile([1, 120], FP32)
    nc.sync.dma_start(wz_sb[:], wz4)
    bias_sb = cpool.tile([M, 1], FP32)
    nc.sync.dma_start(bias_sb[:], biasc)
    gscale_sb = cpool.tile([M, 1], FP32)
    nc.sync.dma_start(gscale_sb[:], gscale)
    id_sb = cpool.tile([128, 128], FP32)
    nc.sync.dma_start(id_sb[:], ident)

    XWG = cpool.tile([H, t_len * 4 * BC], FP32)  # col = t*4BC + g*BC + b
    XAB = cpool.tile([1, t_len * 2 * BC], FP32)  # col = t*2BC + h*BC + b
    LU = cpool.tile([1, NW], FP32)               # col = t*BC + b
    ZB = cpool.tile([1, NW], FP32)
    hx = cpool.tile([H, BC], FP32)
    cx = cpool.tile([H, BC], FP32)
    nc.vector.memset(hx[:], 0.0)
    nc.vector.memset(cx[:], 0.0)
    ln12_sb = cpool.tile([1, 1], FP32)
    nc.vector.memset(ln12_sb[:], LN12)

    # ---- phase 1: LU (single-partition, t-major interleaved) ----
    p1 = ctx.enter_context(tc.tile_pool(name="p1", bufs=1))
    uw = p1.tile([1, NW], FP32)
    nc.sync.dma_start(uw[:], uin.rearrange("b t -> t b"))
    ucl = p1.tile([1, NW], FP32)
    nc.vector.tensor_scalar(ucl[:], uw[:], EPS, 1.0 - EPS, OP.max, OP.min)
    nc.scalar.activation(LU[:], ucl[:], AF.Ln, bias=1.0, scale=-1.0)

    # ---- phase 2: XW in padded-row layout [128, NW] ----
    XW = cpool.tile([M, NW], FP32)
    XW3 = XW[:].rearrange("p (t b) -> p t b", b=BC)
    with (
        tc.tile_pool(name="xraw", bufs=4) as xpool,
        tc.tile_pool(name="xt", bufs=4) as xtpool,
        tc.tile_pool(name="ptr", bufs=3, space="PSUM") as ptpool,
        tc.tile_pool(name="pg", bufs=2, space="PSUM") as pgpool,
    ):
        for b in range(BC):
            for tci in range(n_tc):
                pg = pgpool.tile([M, tchunk], FP32)
                for dc in range(n_dc):
                    xr = xpool.tile([128, 128], FP32)
                    nc.sync.dma_start(
                        xr[:tchunk, :],
                        xin[b, tci * tchunk:(tci + 1) * tchunk,
                            dc * 128:(dc + 1) * 128],
                    )
                    pt = ptpool.tile([128, 128], FP32)
                    nc.tensor.transpose(
                        pt[:, :tchunk], xr[:tchunk, :], id_sb[:tchunk, :tchunk]
                    )
                    xt = xtpool.tile([128, 128], FP32)
                    nc.vector.tensor_copy(xt[:, :tchunk], pt[:, :tchunk])
                    nc.tensor.matmul(
                        pg[:],
                        wx_sb[:, dc * M:(dc + 1) * M],
                        xt[:, :tchunk],
                        start=(dc == 0),
                        stop=(dc == n_dc - 1),
                    )
                # out = gscale * pg + bias  (scaled gate pre-acts + kuma rows)
                nc.scalar.activation(
                    XW3[:, tci * tchunk:(tci + 1) * tchunk, b],
                    pg[:],
                    AF.Identity,
                    bias=bias_sb[:],
                    scale=gscale_sb[:],
                )

    # ---- phase 2.5: DRAM round-trip into loop layouts ----
    dpool = ctx.enter_context(tc.tile_pool(name="scr", bufs=1, space="DRAM"))
    scr = dpool.tile([M, NW], FP32)
    nc.sync.dma_start(scr[:], XW[:])
    # XWG[m, (t g b)] = scr[32g + m, t*BC + b]
    nc.sync.dma_start(
        XWG[:].rearrange("m (t g b) -> m t g b", g=4, b=BC),
        scr[:].rearrange("(gg m) (t b) -> m t gg b", m=32, b=BC)[:H, :, :4, :],
    )
    # XAB[0, (t h b)] = scr[126 + h, t*BC + b]
    nc.sync.dma_start(
        XAB[:].rearrange("p (t h b) -> p t h b", h=2, b=BC),
        scr[126:128, :].rearrange("h (t b) -> t h b", b=BC),
    )

    # ---- phase 3: the sequential loop ----
    pgpool3 = ctx.enter_context(tc.tile_pool(name="pstepg", bufs=4, space="PSUM"))
    pbpool3 = ctx.enter_context(tc.tile_pool(name="pstepb", bufs=4, space="PSUM"))
    sp = ctx.enter_context(tc.tile_pool(name="sstep", bufs=3))
    for t in range(t_len):
        col = slice(t * BC, (t + 1) * BC)
        psB = pbpool3.tile([1, 2 * BC], FP32)
        nc.scalar.activation(
            psB[:], XAB[:, t * 2 * BC:(t + 1) * 2 * BC], AF.Copy
        )
        psG = pgpool3.tile([H, 4 * BC], FP32)
        nc.scalar.activation(
            psG[:], XWG[:, t * 4 * BC:(t + 1) * 4 * BC], AF.Copy
        )
        # kuma pre-acts += [wa_h | wb_h] . hx
        nc.tensor.matmul(
            psB[:, 0:BC], wrec_sb[:, 120:121], hx[:],
            start=False, stop=True, skip_group_check=True,
        )
        nc.tensor.matmul(
            psB[:, BC:2 * BC], wrec_sb[:, 121:122], hx[:],
            start=False, stop=True, skip_group_check=True,
        )
        # gate pre-acts += scaled Whh_g . hx
        for g in range(4):
            nc.tensor.matmul(
                psG[:, g * BC:(g + 1) * BC],
                wrec_sb[:, g * H:(g + 1) * H], hx[:],
                start=False, stop=False, skip_group_check=True,
            )
        # r = max(1/softplus(ab_pre), 0.01)  (in-place on psB, then SBUF)
        nc.scalar.activation(psB[:], psB[:], AF.Exp)
        nc.scalar.activation(psB[:], psB[:], AF.Ln, bias=1.0)
        rab = sp.tile([1, 2 * BC], FP32)
        nc.vector.reciprocal(rab[:], psB[:])
        # z' = clip(1.2 * (1 - (1-u)^rb)^ra, 0.1, 1.1)
        e1i = sp.tile([1, BC], FP32)
        nc.vector.scalar_tensor_tensor(
            e1i[:], rab[:, BC:2 * BC], 0.01, LU[:, col], OP.max, OP.mult
        )
        e1 = sp.tile([1, BC], FP32)
        nc.scalar.activation(e1[:], e1i[:], AF.Exp)
        l2 = sp.tile([1, BC], FP32)
        nc.scalar.activation(l2[:], e1[:], AF.Ln, bias=1.0, scale=-1.0)
        s2 = sp.tile([1, BC], FP32)
        nc.vector.scalar_tensor_tensor(
            s2[:], rab[:, 0:BC], 0.01, l2[:], OP.max, OP.mult
        )
        spt = sp.tile([1, BC], FP32)
        nc.scalar.activation(spt[:], s2[:], AF.Exp, bias=ln12_sb[:])
        nc.vector.tensor_scalar(ZB[:, col], spt[:], 0.1, 1.1, OP.max, OP.min)
        # gates += scaled w_z,g (x) z'
        for g in range(4):
            nc.tensor.matmul(
                psG[:, g * BC:(g + 1) * BC],
                wz_sb[:, g * H:(g + 1) * H], ZB[:, col],
                start=False, stop=True, skip_group_check=True,
            )
        # LSTM cell; pre-acts already sign/scale folded
        ge = sp.tile([H, 4 * BC], FP32)
        nc.scalar.activation(ge[:], psG[:], AF.Exp)
        gd = sp.tile([H, 4 * BC], FP32)
        nc.vector.tensor_scalar_add(gd[:], ge[:], 1.0)
        gr = sp.tile([H, 4 * BC], FP32)
        nc.vector.reciprocal(gr[:], gd[:])
        # sig_i = gr[:,0:BC], sig_f = gr[:,BC:2BC], sig_o = gr[:,2BC:3BC]
        # tanh_g = 1 - 2*gr[:,3BC:4BC]
        tg = sp.tile([H, BC], FP32)
        nc.vector.tensor_scalar(
            tg[:], gr[:, 3 * BC:4 * BC], -2.0, 1.0, OP.mult, OP.add
        )
        t1 = sp.tile([H, BC], FP32)
        nc.vector.tensor_mul(t1[:], gr[:, 0:BC], tg[:])
        t2 = sp.tile([H, BC], FP32)
        nc.vector.tensor_mul(t2[:], gr[:, BC:2 * BC], cx[:])
        nc.vector.tensor_add(cx[:], t1[:], t2[:])
        ce = sp.tile([H, BC], FP32)
        nc.scalar.activation(ce[:], cx[:], AF.Exp, scale=2.0)
        cd = sp.tile([H, BC], FP32)
        nc.vector.tensor_scalar_add(cd[:], ce[:], 1.0)
        cr = sp.tile([H, BC], FP32)
        nc.vector.reciprocal(cr[:], cd[:])
        th = sp.tile([H, BC], FP32)
        nc.vector.tensor_scalar(th[:], cr[:], -2.0, 1.0, OP.mult, OP.add)
        nc.vector.tensor_mul(hx[:], gr[:, 2 * BC:3 * BC], th[:])

    # ---- phase 4: output ----
    zf = cpool.tile([1, NW], FP32)
    nc.vector.tensor_scalar_sub(zf[:], ZB[:], 0.1)
    zf3 = zf[:].rearrange("p (t b) -> p t b", b=BC)
    for b in range(BC):
        nc.sync.dma_start(zout[b:b + 1, :], zf3[:, :, b])


def _build(t_len: int):
    nc = bacc.Bacc(
        "TRN2", target_bir_lowering=False, debug=False, num_devices=NCORES
    )
    io = {
        "xin": nc.dram_tensor("xin", [BC, t_len, D], FP32, kind="ExternalInput").ap(),
        "uin": nc.dram_tensor("uin", [BC, t_len], FP32, kind="ExternalInput").ap(),
        "wxT": nc.dram_tensor("wxT", [D, M], FP32, kind="ExternalInput").ap(),
        "wrecT": nc.dram_tensor("wrecT", [H, 122], FP32, kind="ExternalInput").ap(),
        "wz4": nc.dram_tensor("wz4", [1, 120], FP32, kind="ExternalInput").ap(),
        "biasc": nc.dram_tensor("biasc", [M, 1], FP32, kind="ExternalInput").ap(),
        "gscale": nc.dram_tensor("gscale", [M, 1], FP32, kind="ExternalInput").ap(),
        "ident": nc.dram_tensor("ident", [128, 128], FP32, kind="ExternalInput").ap(),
        "zout": nc.dram_tensor("zout", [BC, t_len], FP32, kind="ExternalOutput").ap(),
    }
    with tile.TileContext(nc) as tc:
        _emit(tc, io, t_len)
    nc.compile()
    return nc


def _prep_weights(Wih, Whh, bih, bhh, Wa, ba, Wb, bb):
    """Host-side (tiny) weight reshuffles; all fp32 numpy."""
    Wih = np.asarray(Wih, np.float32)
    Whh = np.asarray(Whh, np.float32)
    Wa = np.asarray(Wa, np.float32)
    Wb = np.asarray(Wb, np.float32)
    bih = np.asarray(bih, np.float32)
    bhh = np.asarray(bhh, np.float32)

    # phase-2 padded-row layout (groups at 0/32/64/96, kuma at 126/127)
    wx_all = np.zeros((M, D), np.float32)
    bias_all = np.zeros(M, np.float32)
    gscale = np.ones(M, np.float32)
    for g, (src, s) in enumerate(zip(_SRC_GRP, _SCALE_GRP)):
        rows = slice(32 * g, 32 * g + H)
        wx_all[rows] = Wih[src, :D]
        wz_src = Wih[src, D]
        bias_all[rows] = np.float32(s) * (
            bih[src] + bhh[src] - np.float32(0.1) * wz_src
        )
        gscale[rows] = s
    wx_all[126] = Wa[0, :D]
    wx_all[127] = Wb[0, :D]
    bias_all[126] = np.asarray(ba, np.float32)[0]
    bias_all[127] = np.asarray(bb, np.float32)[0]

    # loop weights: scaled Whh_g.T blocks + wa_h + wb_h, and scaled wz
    wrecT = np.zeros((H, 122), np.float32)
    wz4 = np.zeros(120, np.float32)
    for g, (src, s) in enumerate(zip(_SRC_GRP, _SCALE_GRP)):
        wrecT[:, g * H:(g + 1) * H] = np.float32(s) * Whh[src, :].T
        wz4[g * H:(g + 1) * H] = np.float32(s) * Wih[src, D]
    wrecT[:, 120] = Wa[0, D:]
    wrecT[:, 121] = Wb[0, D:]

    ident = np.eye(128, dtype=np.float32)
    return dict(
        wxT=np.ascontiguousarray(wx_all.T), wrecT=wrecT,
        wz4=np.ascontiguousarray(wz4[None, :]),
        biasc=bias_all[:, None], gscale=gscale[:, None], ident=ident,
    )


_CACHED = {}
LAST_RESULTS = None


def _run(inputs: dict, trace: bool = False, t_len: int = T):
    global LAST_RESULTS
    if t_len not in _CACHED:
        _CACHED[t_len] = _build(t_len)
    nc = _CACHED[t_len]
    w = _prep_weights(
        inputs["Wih"], inputs["Whh"], inputs["bih"], inputs["bhh"],
        inputs["Wa"], inputs["ba"], inputs["Wb"], inputs["bb"],
    )
    x = np.asarray(inputs["x"], np.float32)
    u = np.asarray(inputs["u"], np.float32)[..., 0]
    in_maps = []
    for c in range(NCORES):
        m = dict(w)
        m["xin"] = np.ascontiguousarray(x[c * BC:(c + 1) * BC, :t_len])
        m["uin"] = np.ascontiguousarray(u[c * BC:(c + 1) * BC, :t_len])
        in_maps.append(m)
    try:
        res = bass_utils.run_bass_kernel_spmd(
            nc, in_maps, core_ids=list(range(NCORES)), trace=trace
        )
    except ModuleNotFoundError:
        res = bass_utils.run_bass_kernel_spmd(
            nc, in_maps, core_ids=list(range(NCORES)), trace=False
        )
    LAST_RESULTS = res
    out = np.concatenate([r["zout"] for r in res.results], axis=0)
    return out.astype(np.float32)


def kernel(**inputs) -> np.ndarray:
    return _run(inputs, trace=False)

